# revision 1
# baseline (speedup 1.0000x reference)
"""Batched 2048-point DFT on 8 TRN2 NeuronCores — four-step (Cooley-Tukey) version.

n = 2048 = 128 * 16, m = 16*m1 + m2, k = k1 + 128*k2:
  X[b, k1 + 128*k2] = sum_m2 W16[m2,k2] * ( sum_m1 A_m2[m1,k1] * x[b, 16*m1+m2] )
with A_m2[m1,k1] = exp(-2i*pi*(16*m1+m2)*k1/2048)  (128-DFT with the inter-stage
twiddle folded in — no separate twiddle pass).

Per core (batch shard of 512):
  stage 1: 16 matmuls pairs, stationary A_m2 (f32r), moving xR[m1,(m2,b)]
           -> Z[k1, (b, m2)] in SBUF (m2 innermost)
  stage 2: PE transposes of 64x2 [128,128] chunks -> T[(b_lo,m2), k1]
  stage 3: block-diag stationary S[(bl,m2),(bl,k2)] = W16[m2,k2] matmuls
           -> X[b, k] directly (4D-AP DMA scatter, no host transpose of output)
All heavy host prep (reshape/negation/twiddle build) is free host-side work.
"""

import sys

for _p in ("/opt/trn_rl_repo", "/root/.axon_site/_ro/trn_rl_repo"):
    if _p not in sys.path:
        sys.path.insert(0, _p)

import numpy as np

import concourse.bass as bass
import concourse.mybir as mybir
import concourse.tile as tile
from concourse import bacc
from concourse.bass_utils import run_bass_kernel_spmd
from concourse.masks import make_identity

BATCH = 4096
NFFT = 2048
NCORES = 8
BPC = BATCH // NCORES  # 512
N1 = 128
N2 = 16
NCHUNK = BPC * N2 // 128  # 64 transpose chunks per part
NGRP = NCHUNK // 4  # 16 stage-3 groups

F32 = mybir.dt.float32
F32R = mybir.dt.float32r

_CACHE = {}


def _build_nc():
    nc = bacc.Bacc("TRN2", target_bir_lowering=False, debug=False)

    # xR layout: [m2 16, m1 128, b 512] flat [2048, 512] (contiguous per-q DMA)
    xre_d = nc.dram_tensor("xre", [N2 * N1, BPC], F32, kind="ExternalInput").ap()
    xim_d = nc.dram_tensor("xim", [N2 * N1, BPC], F32, kind="ExternalInput").ap()
    # A stationaries: [m2, m1 128, 3(re, im, imneg), k1 128] flat [2048, 384]
    a_d = nc.dram_tensor("amat", [N2 * 128, 3 * 128], F32, kind="ExternalInput").ap()
    # S block-diag: [3, 128, 128]
    s_d = nc.dram_tensor("smat", [3 * 128, 128], F32, kind="ExternalInput").ap()
    ore_d = nc.dram_tensor("ore", [BPC, NFFT], F32, kind="ExternalOutput").ap()
    oim_d = nc.dram_tensor("oim", [BPC, NFFT], F32, kind="ExternalOutput").ap()

    a_v = a_d.bitcast(F32R).rearrange("(q p) (v k) -> q p v k", q=N2, v=3)
    s_v = s_d.bitcast(F32R).rearrange("(v p) k -> v p k", v=3)

    with tile.TileContext(nc) as tc:
        with (
            tc.tile_pool(name="const", bufs=1) as cpool,
            tc.tile_pool(name="x", bufs=1) as xpool,
            tc.tile_pool(name="z", bufs=1) as zpool,
            tc.tile_pool(name="t", bufs=3) as tpool,
            tc.tile_pool(name="o", bufs=3) as opool,
            tc.tile_pool(name="ps1", bufs=3, space="PSUM") as ps1pool,
            tc.tile_pool(name="pst", bufs=3, space="PSUM") as pstpool,
            tc.tile_pool(name="ps2", bufs=2, space="PSUM") as ps2pool,
        ):
            # constants + resident moving operand, interleaved for startup
            a_t = cpool.tile([128, N2, 3, 128], F32R, tag="amat")
            xre_t = xpool.tile([128, N2, BPC], F32R, tag="xre")
            xim_t = xpool.tile([128, N2, BPC], F32R, tag="xim")
            xre_v = xre_d.bitcast(F32R).rearrange("(q p) b -> q p b", q=N2)
            xim_v = xim_d.bitcast(F32R).rearrange("(q p) b -> q p b", q=N2)
            for q in range(N2):
                nc.sync.dma_start(a_t[:, q, :, :], a_v[q])
                nc.sync.dma_start(xre_t[:, q, :], xre_v[q])
                nc.sync.dma_start(xim_t[:, q, :], xim_v[q])
            s_t = cpool.tile([128, 3, 128], F32R, tag="smat")
            nc.sync.dma_start(s_t[:], s_v.transpose([1, 0, 2]))
            ident = cpool.tile([128, 128], F32, tag="ident")
            make_identity(nc, ident[:])

            # stage-1 output, col = b*16 + m2 (m2 innermost: transpose chunks
            # contiguous, stage-1 psum copies strided by 16)
            z_re = zpool.tile([128, BPC * N2], F32, tag="zre")
            z_im = zpool.tile([128, BPC * N2], F32, tag="zim")
            z_re3 = z_re[:].rearrange("p (b q) -> p b q", q=N2)
            z_im3 = z_im[:].rearrange("p (b q) -> p b q", q=N2)

            # ---- stage 1 ----
            for q in range(N2):
                ps_re = ps1pool.tile([128, BPC], F32, tag="ps1")
                ps_im = ps1pool.tile([128, BPC], F32, tag="ps1")
                are = a_t[:, q, 0, :]
                aim = a_t[:, q, 1, :]
                aimn = a_t[:, q, 2, :]
                nc.tensor.matmul(ps_re[:], are, xre_t[:, q, :], start=True, stop=False)
                nc.tensor.matmul(ps_im[:], are, xim_t[:, q, :], start=True, stop=False)
                nc.tensor.matmul(ps_im[:], aim, xre_t[:, q, :], start=False, stop=True)
                nc.tensor.matmul(ps_re[:], aimn, xim_t[:, q, :], start=False, stop=True)
                nc.vector.tensor_copy(z_re3[:, :, q], ps_re[:])
                nc.scalar.copy(z_im3[:, :, q], ps_im[:])

            # ---- stage 2 + 3, per group of 4 chunks ----
            sre = s_t[:, 0, :]
            sim = s_t[:, 1, :]
            simn = s_t[:, 2, :]
            for g in range(NGRP):
                t_re = tpool.tile([128, 512], F32R, tag="tre")
                t_im = tpool.tile([128, 512], F32R, tag="tim")
                pt_re = pstpool.tile([128, 512], F32, tag="pt")
                pt_im = pstpool.tile([128, 512], F32, tag="pt")
                for j in range(4):
                    c = g * 4 + j
                    csl = slice(c * 128, (c + 1) * 128)
                    jsl = slice(j * 128, (j + 1) * 128)
                    nc.tensor.transpose(pt_re[:, jsl], z_re[:, csl], ident[:])
                    nc.tensor.transpose(pt_im[:, jsl], z_im[:, csl], ident[:])
                nc.vector.tensor_copy(t_re[:], pt_re[:])
                nc.scalar.copy(t_im[:], pt_im[:])

                ps2_re = ps2pool.tile([128, 512], F32, tag="ps2")
                ps2_im = ps2pool.tile([128, 512], F32, tag="ps2")
                nc.tensor.matmul(ps2_re[:], sre, t_re[:], start=True, stop=False)
                nc.tensor.matmul(ps2_im[:], sre, t_im[:], start=True, stop=False)
                nc.tensor.matmul(ps2_im[:], sim, t_re[:], start=False, stop=True)
                nc.tensor.matmul(ps2_re[:], simn, t_im[:], start=False, stop=True)

                o_re = opool.tile([128, 512], F32, tag="ore")
                o_im = opool.tile([128, 512], F32, tag="oim")
                nc.vector.tensor_copy(o_re[:], ps2_re[:])
                nc.scalar.copy(o_im[:], ps2_im[:])

                # scatter: partition p=(bl,kt), col=(j,ko); b = g*32+j*8+bl, k = kt*128+ko
                dst_re = ore_d.rearrange(
                    "(g j bl) (kt ko) -> g (bl kt) j ko", g=NGRP, j=4, bl=8, kt=N2
                )[g]
                dst_im = oim_d.rearrange(
                    "(g j bl) (kt ko) -> g (bl kt) j ko", g=NGRP, j=4, bl=8, kt=N2
                )[g]
                nc.sync.dma_start(dst_re, o_re[:].rearrange("p (j ko) -> p j ko", j=4))
                nc.sync.dma_start(dst_im, o_im[:].rearrange("p (j ko) -> p j ko", j=4))

    nc.compile()
    return nc


def _consts():
    m1 = np.arange(N1, dtype=np.float64)
    k1 = np.arange(N1, dtype=np.float64)
    m2 = np.arange(N2, dtype=np.float64)
    k2 = np.arange(N2, dtype=np.float64)
    # A_m2[m1,k1] = exp(-2i pi (16 m1 + m2) k1 / 2048)
    a = np.empty((N2, 3, N1, N1), np.float32)
    for q in range(N2):
        ph = -2.0 * np.pi * np.outer(16.0 * m1 + q, k1) / NFFT
        a[q, 0] = np.cos(ph).astype(np.float32)
        a[q, 1] = np.sin(ph).astype(np.float32)
        a[q, 2] = -a[q, 1]
    # S[(bl,m2),(bl,k2)] = W16[m2,k2]
    ph16 = -2.0 * np.pi * np.outer(m2, k2) / N2
    w16re = np.cos(ph16).astype(np.float32)
    w16im = np.sin(ph16).astype(np.float32)
    s = np.zeros((3, 128, 128), np.float32)
    for bl in range(8):
        sl = slice(bl * 16, (bl + 1) * 16)
        s[0][sl, sl] = w16re
        s[1][sl, sl] = w16im
        s[2][sl, sl] = -w16im
    return (
        np.ascontiguousarray(a.transpose(0, 2, 1, 3).reshape(N2 * 128, 3 * 128)),
        np.ascontiguousarray(s.reshape(3 * 128, 128)),
    )


def run(signal_re, signal_im, trace=False, tmpdir=None):
    if "nc" not in _CACHE:
        _CACHE["nc"] = _build_nc()
        _CACHE["c"] = _consts()
    nc = _CACHE["nc"]
    amat, smat = _CACHE["c"]

    sre = np.asarray(signal_re, dtype=np.float32)
    sim = np.asarray(signal_im, dtype=np.float32)

    in_maps = []
    for c in range(NCORES):
        bsl = slice(c * BPC, (c + 1) * BPC)
        # xR[m1, m2, b]
        xre = np.ascontiguousarray(
            sre[bsl].reshape(BPC, N1, N2).transpose(2, 1, 0).reshape(N2 * N1, BPC)
        )
        xim = np.ascontiguousarray(
            sim[bsl].reshape(BPC, N1, N2).transpose(2, 1, 0).reshape(N2 * N1, BPC)
        )
        in_maps.append({"xre": xre, "xim": xim, "amat": amat, "smat": smat})

    # first execution of a fresh NEFF occasionally fails with a transient
    # INTERNAL runtime error; retry a couple of times before giving up
    last_exc = None
    for attempt in range(3):
        try:
            br = run_bass_kernel_spmd(
                nc, in_maps, list(range(NCORES)), trace=trace, tmpdir=tmpdir
            )
            break
        except Exception as e:
            last_exc = e
            import time

            time.sleep(2.0)
    else:
        raise last_exc

    out_re = np.empty((BATCH, NFFT), np.float32)
    out_im = np.empty((BATCH, NFFT), np.float32)
    for c in range(NCORES):
        bsl = slice(c * BPC, (c + 1) * BPC)
        out_re[bsl, :] = br.results[c]["ore"]
        out_im[bsl, :] = br.results[c]["oim"]
    return (out_re, out_im), br


def kernel(signal_re, signal_im):
    return run(signal_re, signal_im)[0]



# revision 2
# speedup vs baseline: 1.1288x; 1.1288x over previous
"""Batched 2048-point DFT on 8 TRN2 NeuronCores — fp16 four-step version.

n = 2048 = 128 * 16, m = 16*m1 + m2, k = k1 + 128*k2:
  X[b, k1 + 128*k2] = sum_m2 W16[m2,k2] * ( sum_m1 A_m2[m1,k1] * x[b, 16*m1+m2] )
with A_m2[m1,k1] = exp(-2i*pi*(16*m1+m2)*k1/2048).

All operands fp16 (tolerance 2e-2 >> fp16 error ~1e-3): halves HBM traffic,
enables FWL fast weight loads, 1 cyc/row transposes, and 2x DVE reads of the
fp16 transpose PSUM output.  PSUM accumulation stays fp32 (TRN2 requirement).

Per core (batch shard of 512):
  warmup: dummy matmuls during the input DMA fill to lift the PE HAM throttle
  stage 1: per q: 4 fp16 matmuls acc into fp32 PSUM -> Z[k1,(b,m2)] fp16 SBUF
  stage 2: PE fp16 transposes, 8 per group into one [128,1024] fp16 PSUM bank
           -> one DVE/ACT evac per group into T[(bl,m2),(re|im)(j,k1)]
  stage 3: block-diag S = I8 (x) W16 matmuls -> o[(bl,k2),(re|im)(j,k1)] fp16
           -> contiguous DMA dump, host-side unscramble (free)
"""

import sys

for _p in ("/opt/trn_rl_repo", "/root/.axon_site/_ro/trn_rl_repo"):
    if _p not in sys.path:
        sys.path.insert(0, _p)

import numpy as np

import concourse.bass as bass
import concourse.mybir as mybir
import concourse.tile as tile
from concourse import bacc
from concourse.bass_utils import run_bass_kernel_spmd
from concourse.masks import make_identity

BATCH = 4096
NFFT = 2048
NCORES = 8
BPC = BATCH // NCORES  # 512
N1 = 128
N2 = 16
NCHUNK = BPC * N2 // 128  # 64 transpose chunks per core
NGRP = NCHUNK // 4  # 16 stage-3 groups
NWARM = 12  # matmuls to lift the HAM throttle during input DMA

F32 = mybir.dt.float32
F16 = mybir.dt.float16

_CACHE = {}


def _build_nc():
    nc = bacc.Bacc("TRN2", target_bir_lowering=False, debug=False)

    # xR layout: [m2 16, m1 128, b 512] flat [2048, 512]
    xre_d = nc.dram_tensor("xre", [N2 * N1, BPC], F16, kind="ExternalInput").ap()
    xim_d = nc.dram_tensor("xim", [N2 * N1, BPC], F16, kind="ExternalInput").ap()
    # A stationaries: [m2, m1 128, 3(re, im, imneg), k1 128] flat [2048, 384]
    a_d = nc.dram_tensor("amat", [N2 * N1, 3 * 128], F16, kind="ExternalInput").ap()
    # S block-diag: [3, 128, 128]
    s_d = nc.dram_tensor("smat", [3 * 128, 128], F16, kind="ExternalInput").ap()
    # output dump: per group g: [128 (bl,k2), 1024 (re|im, j, k1)]
    o_d = nc.dram_tensor("odump", [NGRP * 128, 1024], F16, kind="ExternalOutput").ap()

    a_v = a_d.rearrange("(Q q p) (v k) -> Q p q v k", Q=4, q=4, v=3)
    xre_v = xre_d.rearrange("(Q q p) b -> Q p q b", Q=4, q=4)
    xim_v = xim_d.rearrange("(Q q p) b -> Q p q b", Q=4, q=4)
    s_v = s_d.rearrange("(v p) k -> v p k", v=3)
    o_v = o_d.rearrange("(g p) c -> g p c", g=NGRP)

    with tile.TileContext(nc) as tc:
        with (
            tc.tile_pool(name="const", bufs=1) as cpool,
            tc.tile_pool(name="x", bufs=1) as xpool,
            tc.tile_pool(name="z", bufs=1) as zpool,
            tc.tile_pool(name="t", bufs=3) as tpool,
            tc.tile_pool(name="o", bufs=3) as opool,
            tc.tile_pool(name="psA", bufs=5, space="PSUM") as psA,
            tc.tile_pool(name="pst", bufs=3, space="PSUM") as pstpool,
        ):
            # identity first: used by PE warmup matmuls during the DMA fill
            ident = cpool.tile([128, 128], F16, tag="ident")
            make_identity(nc, ident[:])

            a_t = cpool.tile([128, N2, 3, 128], F16, tag="amat")
            xre_t = xpool.tile([128, N2, BPC], F16, tag="xre")
            xim_t = xpool.tile([128, N2, BPC], F16, tag="xim")
            for Q in range(4):
                qsl = slice(Q * 4, (Q + 1) * 4)
                nc.sync.dma_start(a_t[:, qsl], a_v[Q])
                nc.sync.dma_start(xre_t[:, qsl], xre_v[Q])
                nc.sync.dma_start(xim_t[:, qsl], xim_v[Q])
            s_t = cpool.tile([128, 3, 128], F16, tag="smat")
            nc.sync.dma_start(s_t[:], s_v.transpose([1, 0, 2]))

            # HAM warmup: dead matmuls on the identity while inputs stream in
            for _ in range(NWARM):
                psw = psA.tile([128, 512], F32, tag="ps")
                for rep in range(4):
                    nc.tensor.matmul(
                        psw[:, rep * 128 : (rep + 1) * 128],
                        ident[:],
                        ident[:],
                        start=True,
                        stop=True,
                    )

            # stage-1 output, col = b*16 + m2 (m2 innermost: transpose chunks
            # contiguous; stage-1 evacuation writes strided by 16)
            z_re = zpool.tile([128, BPC * N2], F16, tag="zre")
            z_im = zpool.tile([128, BPC * N2], F16, tag="zim")
            z_re3 = z_re[:].rearrange("p (b q) -> p b q", q=N2)
            z_im3 = z_im[:].rearrange("p (b q) -> p b q", q=N2)

            # ---- stage 1 ----
            for q in range(N2):
                ps_re = psA.tile([128, BPC], F32, tag="ps")
                ps_im = psA.tile([128, BPC], F32, tag="ps")
                are = a_t[:, q, 0, :]
                aim = a_t[:, q, 1, :]
                aimn = a_t[:, q, 2, :]
                nc.tensor.matmul(ps_re[:], are, xre_t[:, q, :], start=True, stop=False)
                nc.tensor.matmul(ps_im[:], are, xim_t[:, q, :], start=True, stop=False)
                nc.tensor.matmul(ps_im[:], aim, xre_t[:, q, :], start=False, stop=True)
                nc.tensor.matmul(ps_re[:], aimn, xim_t[:, q, :], start=False, stop=True)
                nc.vector.tensor_copy(z_re3[:, :, q], ps_re[:])
                nc.scalar.copy(z_im3[:, :, q], ps_im[:])

            # ---- stage 2 + 3, software-pipelined by one group ----
            sre = s_t[:, 0, :]
            sim = s_t[:, 1, :]
            simn = s_t[:, 2, :]

            def emit_tp(g):
                # 8 fp16 transposes into one [128,1024] fp16 PSUM bank
                pt = pstpool.tile([128, 1024], F16, tag="pt")
                for j in range(4):
                    c = g * 4 + j
                    csl = slice(c * 128, (c + 1) * 128)
                    nc.tensor.transpose(
                        pt[:, j * 128 : (j + 1) * 128], z_re[:, csl], ident[:]
                    )
                    nc.tensor.transpose(
                        pt[:, 512 + j * 128 : 512 + (j + 1) * 128],
                        z_im[:, csl],
                        ident[:],
                    )
                # single evacuation of the whole bank (fp16 2x on DVE)
                t_t = tpool.tile([128, 1024], F16, tag="t")
                if g % 4 < 3:
                    nc.vector.tensor_copy(t_t[:], pt[:])
                else:
                    nc.scalar.copy(t_t[:], pt[:])
                return t_t

            def emit_s3(g, t_t):
                t_re = t_t[:, 0:512]
                t_im = t_t[:, 512:1024]
                ps2_re = psA.tile([128, BPC], F32, tag="ps")
                ps2_im = psA.tile([128, BPC], F32, tag="ps")
                nc.tensor.matmul(ps2_re[:], sre, t_re, start=True, stop=False)
                nc.tensor.matmul(ps2_im[:], sre, t_im, start=True, stop=False)
                nc.tensor.matmul(ps2_im[:], sim, t_re, start=False, stop=True)
                nc.tensor.matmul(ps2_re[:], simn, t_im, start=False, stop=True)
                o_t = opool.tile([128, 1024], F16, tag="o")
                nc.vector.tensor_copy(o_t[:, 0:512], ps2_re[:])
                nc.scalar.copy(o_t[:, 512:1024], ps2_im[:])
                nc.sync.dma_start(o_v[g], o_t[:])

            t_prev = emit_tp(0)
            for g in range(NGRP):
                t_next = emit_tp(g + 1) if g + 1 < NGRP else None
                emit_s3(g, t_prev)
                t_prev = t_next

    nc.compile()
    return nc


def _consts():
    m1 = np.arange(N1, dtype=np.float64)
    k1 = np.arange(N1, dtype=np.float64)
    m2 = np.arange(N2, dtype=np.float64)
    k2 = np.arange(N2, dtype=np.float64)
    # A_m2[m1,k1] = exp(-2i pi (16 m1 + m2) k1 / 2048)
    a = np.empty((N2, 3, N1, N1), np.float16)
    for q in range(N2):
        ph = -2.0 * np.pi * np.outer(16.0 * m1 + q, k1) / NFFT
        a[q, 0] = np.cos(ph).astype(np.float16)
        a[q, 1] = np.sin(ph).astype(np.float16)
        a[q, 2] = -a[q, 1]
    # S[(bl,m2),(bl,k2)] = W16[m2,k2]
    ph16 = -2.0 * np.pi * np.outer(m2, k2) / N2
    w16re = np.cos(ph16).astype(np.float16)
    w16im = np.sin(ph16).astype(np.float16)
    s = np.zeros((3, 128, 128), np.float16)
    for bl in range(8):
        sl = slice(bl * 16, (bl + 1) * 16)
        s[0][sl, sl] = w16re
        s[1][sl, sl] = w16im
        s[2][sl, sl] = -w16im
    return (
        np.ascontiguousarray(a.transpose(0, 2, 1, 3).reshape(N2 * 128, 3 * 128)),
        np.ascontiguousarray(s.reshape(3 * 128, 128)),
    )


def run(signal_re, signal_im, trace=False, tmpdir=None):
    if "nc" not in _CACHE:
        _CACHE["nc"] = _build_nc()
        _CACHE["c"] = _consts()
    nc = _CACHE["nc"]
    amat, smat = _CACHE["c"]

    sre = np.asarray(signal_re, dtype=np.float32).astype(np.float16)
    sim = np.asarray(signal_im, dtype=np.float32).astype(np.float16)

    in_maps = []
    for c in range(NCORES):
        bsl = slice(c * BPC, (c + 1) * BPC)
        # xR[m2, m1, b]
        xre = np.ascontiguousarray(
            sre[bsl].reshape(BPC, N1, N2).transpose(2, 1, 0).reshape(N2 * N1, BPC)
        )
        xim = np.ascontiguousarray(
            sim[bsl].reshape(BPC, N1, N2).transpose(2, 1, 0).reshape(N2 * N1, BPC)
        )
        in_maps.append({"xre": xre, "xim": xim, "amat": amat, "smat": smat})

    # first execution of a fresh NEFF occasionally fails with a transient
    # INTERNAL runtime error; retry a couple of times before giving up
    last_exc = None
    for attempt in range(3):
        try:
            br = run_bass_kernel_spmd(
                nc, in_maps, list(range(NCORES)), trace=trace, tmpdir=tmpdir
            )
            break
        except Exception as e:
            last_exc = e
            import time

            time.sleep(2.0)
    else:
        raise last_exc

    out_re = np.empty((BATCH, NFFT), np.float32)
    out_im = np.empty((BATCH, NFFT), np.float32)
    for c in range(NCORES):
        bsl = slice(c * BPC, (c + 1) * BPC)
        # dump[g*128+p, col]: p=(bl,kt), col=(reim, j, ko); b=g*32+j*8+bl,
        # k = kt*128 + ko
        d = br.results[c]["odump"].reshape(NGRP, 8, N2, 2, 4, 128)
        arr = d.transpose(3, 0, 4, 1, 2, 5).reshape(2, BPC, NFFT).astype(np.float32)
        out_re[bsl, :] = arr[0]
        out_im[bsl, :] = arr[1]
    return (out_re, out_im), br


def kernel(signal_re, signal_im):
    return run(signal_re, signal_im)[0]


# revision 8
# speedup vs baseline: 1.4681x; 1.3005x over previous
"""Batched 2048-point DFT on 8 TRN2 NeuronCores — fp16 four-step version.

n = 2048 = 128 * 16, m = 16*m1 + m2, k = k1 + 128*k2:
  X[b, k1 + 128*k2] = sum_m2 W16[m2,k2] * ( sum_m1 A_m2[m1,k1] * x[b, 16*m1+m2] )
with A_m2[m1,k1] = exp(-2i*pi*(16*m1+m2)*k1/2048).

All operands fp16 (tolerance 2e-2 >> fp16 error ~1e-3): halves HBM traffic,
enables FWL fast weight loads, 1 cyc/row transposes, and 2x DVE reads of the
fp16 transpose PSUM output.  PSUM accumulation stays fp32 (TRN2 requirement).

Per core (batch shard of 512):
  warmup: dummy matmuls during the input DMA fill to lift the PE HAM throttle
  stage 1: per q: 4 fp16 matmuls acc into fp32 PSUM -> Z[k1,(b,m2)] fp16 SBUF
  stage 2: PE fp16 transposes, 8 per group into one [128,1024] fp16 PSUM bank
           -> one DVE/ACT evac per group into T[(bl,m2),(re|im)(j,k1)]
  stage 3: block-diag S = I8 (x) W16 matmuls -> o[(bl,k2),(re|im)(j,k1)] fp16
           -> contiguous DMA dump, host-side unscramble (free)
"""

import sys

for _p in ("/opt/trn_rl_repo", "/root/.axon_site/_ro/trn_rl_repo"):
    if _p not in sys.path:
        sys.path.insert(0, _p)

import numpy as np

import concourse.bass as bass
import concourse.mybir as mybir
import concourse.tile as tile
from concourse import bacc
from concourse.bass_utils import run_bass_kernel_spmd
from concourse.masks import make_identity

BATCH = 4096
NFFT = 2048
NCORES = 8
BPC = BATCH // NCORES  # 512
N1 = 128
N2 = 16
NCHUNK = BPC * N2 // 128  # 64 transpose chunks per core
NGRP = NCHUNK // 4  # 16 stage-3 groups
NWARM = 12  # matmuls to lift the HAM throttle during input DMA

F32 = mybir.dt.float32
F16 = mybir.dt.float16

_CACHE = {}


def _build_nc():
    nc = bacc.Bacc("TRN2", target_bir_lowering=False, debug=False)

    # xR layout: [m2 16, m1 128, b 512] flat [2048, 512]
    xre_d = nc.dram_tensor("xre", [N2 * N1, BPC], F16, kind="ExternalInput").ap()
    xim_d = nc.dram_tensor("xim", [N2 * N1, BPC], F16, kind="ExternalInput").ap()
    # A stationaries: [m2, m1 128, 3(re, im, imneg), k1 128] flat [2048, 384]
    a_d = nc.dram_tensor("amat", [N2 * N1, 3 * 128], F16, kind="ExternalInput").ap()
    # S block-diag: [3, 128, 128]
    s_d = nc.dram_tensor("smat", [3 * 128, 128], F16, kind="ExternalInput").ap()
    # output dump: per group g: [128 (bl,k2), 1024 (re|im, j, k1)]
    o_d = nc.dram_tensor("odump", [NGRP * 128, 1024], F16, kind="ExternalOutput").ap()

    a_v = a_d.rearrange("(Q q p) (v k) -> Q p q v k", Q=4, q=4, v=3)
    xre_v = xre_d.rearrange("(Q q p) b -> Q p q b", Q=4, q=4)
    xim_v = xim_d.rearrange("(Q q p) b -> Q p q b", Q=4, q=4)
    s_v = s_d.rearrange("(v p) k -> v p k", v=3)
    o_v = o_d.rearrange("(g p) c -> g p c", g=NGRP)

    with tile.TileContext(nc) as tc:
        with (
            tc.tile_pool(name="const", bufs=1) as cpool,
            tc.tile_pool(name="x", bufs=1) as xpool,
            tc.tile_pool(name="z", bufs=1) as zpool,
            tc.tile_pool(name="t", bufs=3) as tpool,
            tc.tile_pool(name="o", bufs=3) as opool,
        ):
            # identity first: used by PE warmup matmuls during the DMA fill
            ident = cpool.tile([128, 128], F16, tag="ident")
            make_identity(nc, ident[:])

            a_t = cpool.tile([128, N2, 3, 128], F16, tag="amat")
            xre_t = xpool.tile([128, N2, BPC], F16, tag="xre")
            xim_t = xpool.tile([128, N2, BPC], F16, tag="xim")
            for Q in range(4):
                qsl = slice(Q * 4, (Q + 1) * 4)
                nc.sync.dma_start(a_t[:, qsl], a_v[Q])
                nc.sync.dma_start(xre_t[:, qsl], xre_v[Q])
                nc.sync.dma_start(xim_t[:, qsl], xim_v[Q])
            s_t = cpool.tile([128, 3, 128], F16, tag="smat")
            nc.sync.dma_start(s_t[:], s_v.transpose([1, 0, 2]))

            # stage-1 output, col = b2*32 + q*2 + b0 (b = 2*b2 + b0): the
            # evacuation writes adjacent fp16 PAIRS (4B-aligned words, the
            # fast strided-write path), transpose chunks stay contiguous,
            # and the resulting permuted t-partition order is absorbed by a
            # host-permuted S and output unscramble (both free).
            z_re = zpool.tile([128, BPC * N2], F16, tag="zre")
            z_im = zpool.tile([128, BPC * N2], F16, tag="zim")
            z_re4 = z_re[:].rearrange("p (c q b) -> p c q b", q=N2, b=2)
            z_im4 = z_im[:].rearrange("p (c q b) -> p c q b", q=N2, b=2)

            with tc.tile_pool(name="ps1", bufs=6, space="PSUM") as ps1pool:
                # HAM warmup: dead matmuls on the identity while inputs stream
                for _ in range(NWARM):
                    psw = ps1pool.tile([128, 512], F32, tag="ps")
                    for rep in range(4):
                        nc.tensor.matmul(
                            psw[:, rep * 128 : (rep + 1) * 128],
                            ident[:],
                            ident[:],
                            start=True,
                            stop=True,
                        )

                # ---- stage 1 ----
                for q in range(N2):
                    ps_re = ps1pool.tile([128, BPC], F32, tag="ps")
                    ps_im = ps1pool.tile([128, BPC], F32, tag="ps")
                    are = a_t[:, q, 0, :]
                    aim = a_t[:, q, 1, :]
                    aimn = a_t[:, q, 2, :]
                    nc.tensor.matmul(
                        ps_re[:], are, xre_t[:, q, :], start=True, stop=False
                    )
                    nc.tensor.matmul(
                        ps_im[:], are, xim_t[:, q, :], start=True, stop=False
                    )
                    nc.tensor.matmul(
                        ps_im[:], aim, xre_t[:, q, :], start=False, stop=True
                    )
                    nc.tensor.matmul(
                        ps_re[:], aimn, xim_t[:, q, :], start=False, stop=True
                    )
                    nc.vector.tensor_copy(z_re4[:, :, q, :], ps_re[:])
                    nc.scalar.copy(z_im4[:, :, q, :], ps_im[:])

            # ---- stage 2 + 3, software-pipelined by one group ----
            sre = s_t[:, 0, :]
            sim = s_t[:, 1, :]
            simn = s_t[:, 2, :]

            with (
                tc.tile_pool(name="pst", bufs=3, space="PSUM") as pstpool,
                tc.tile_pool(name="ps3", bufs=2, space="PSUM") as ps3pool,
            ):

                def emit_tp(g):
                    # 8 fp16 transposes into one [128,1024] fp16 PSUM bank
                    pt = pstpool.tile([128, 1024], F16, tag="pt")
                    for j in range(4):
                        c = g * 4 + j
                        csl = slice(c * 128, (c + 1) * 128)
                        nc.tensor.transpose(
                            pt[:, j * 128 : (j + 1) * 128], z_re[:, csl], ident[:]
                        )
                        nc.tensor.transpose(
                            pt[:, 512 + j * 128 : 512 + (j + 1) * 128],
                            z_im[:, csl],
                            ident[:],
                        )
                    # single evacuation of the whole bank (fp16 2x on DVE)
                    t_t = tpool.tile([128, 1024], F16, tag="t")
                    if g % 8 < 5:
                        nc.vector.tensor_copy(t_t[:], pt[:])
                    else:
                        nc.scalar.copy(t_t[:], pt[:])
                    return t_t

                def emit_s3(g, t_t):
                    t_re = t_t[:, 0:512]
                    t_im = t_t[:, 512:1024]
                    # re and im in one 2-bank tile -> single wide evacuation
                    ps2 = ps3pool.tile([128, 1024], F32, tag="ps3")
                    ps2_re = ps2[:, 0:512]
                    ps2_im = ps2[:, 512:1024]
                    nc.tensor.matmul(ps2_re, sre, t_re, start=True, stop=False)
                    nc.tensor.matmul(ps2_im, sre, t_im, start=True, stop=False)
                    nc.tensor.matmul(ps2_im, sim, t_re, start=False, stop=True)
                    nc.tensor.matmul(ps2_re, simn, t_im, start=False, stop=True)
                    o_t = opool.tile([128, 1024], F16, tag="o")
                    if g % 2 == 0:
                        nc.vector.tensor_copy(o_t[:], ps2[:])
                    else:
                        nc.scalar.copy(o_t[:], ps2[:])
                    nc.sync.dma_start(o_v[g], o_t[:])

                t_prev = emit_tp(0)
                for g in range(NGRP):
                    t_next = emit_tp(g + 1) if g + 1 < NGRP else None
                    emit_s3(g, t_prev)
                    t_prev = t_next

    nc.compile()
    return nc


def _consts():
    m1 = np.arange(N1, dtype=np.float64)
    k1 = np.arange(N1, dtype=np.float64)
    m2 = np.arange(N2, dtype=np.float64)
    k2 = np.arange(N2, dtype=np.float64)
    # A_m2[m1,k1] = exp(-2i pi (16 m1 + m2) k1 / 2048)
    a = np.empty((N2, 3, N1, N1), np.float16)
    for q in range(N2):
        ph = -2.0 * np.pi * np.outer(16.0 * m1 + q, k1) / NFFT
        a[q, 0] = np.cos(ph).astype(np.float16)
        a[q, 1] = np.sin(ph).astype(np.float16)
        a[q, 2] = -a[q, 1]
    # permuted block-diag S for the pair-interleaved t-partition order:
    # partition p = b2*32 + i*2 + b0 (bl = 2*b2+b0; i = m2 on rows, k2 on
    # cols); nonzero iff row bl == col bl
    ph16 = -2.0 * np.pi * np.outer(m2, k2) / N2
    w16re = np.cos(ph16).astype(np.float16)
    w16im = np.sin(ph16).astype(np.float16)
    p = np.arange(128)
    blp = (p // 32) * 2 + (p % 2)
    ip = (p % 32) // 2
    mask = (blp[:, None] == blp[None, :]).astype(np.float16)
    s = np.zeros((3, 128, 128), np.float16)
    s[0] = w16re[np.ix_(ip, ip)] * mask
    s[1] = w16im[np.ix_(ip, ip)] * mask
    s[2] = -s[1]
    return (
        np.ascontiguousarray(a.transpose(0, 2, 1, 3).reshape(N2 * 128, 3 * 128)),
        np.ascontiguousarray(s.reshape(3 * 128, 128)),
    )


def run(signal_re, signal_im, trace=False, tmpdir=None):
    if "nc" not in _CACHE:
        _CACHE["nc"] = _build_nc()
        _CACHE["c"] = _consts()
    nc = _CACHE["nc"]
    amat, smat = _CACHE["c"]

    sre = np.asarray(signal_re, dtype=np.float32).astype(np.float16)
    sim = np.asarray(signal_im, dtype=np.float32).astype(np.float16)

    in_maps = []
    for c in range(NCORES):
        bsl = slice(c * BPC, (c + 1) * BPC)
        # xR[m2, m1, b]
        xre = np.ascontiguousarray(
            sre[bsl].reshape(BPC, N1, N2).transpose(2, 1, 0).reshape(N2 * N1, BPC)
        )
        xim = np.ascontiguousarray(
            sim[bsl].reshape(BPC, N1, N2).transpose(2, 1, 0).reshape(N2 * N1, BPC)
        )
        in_maps.append({"xre": xre, "xim": xim, "amat": amat, "smat": smat})

    # first execution of a fresh NEFF occasionally fails with a transient
    # INTERNAL runtime error; retry a couple of times before giving up
    last_exc = None
    for attempt in range(3):
        try:
            br = run_bass_kernel_spmd(
                nc, in_maps, list(range(NCORES)), trace=trace, tmpdir=tmpdir
            )
            break
        except Exception as e:
            last_exc = e
            import time

            time.sleep(2.0)
    else:
        raise last_exc

    out_re = np.empty((BATCH, NFFT), np.float32)
    out_im = np.empty((BATCH, NFFT), np.float32)
    for c in range(NCORES):
        bsl = slice(c * BPC, (c + 1) * BPC)
        # dump[g*128+p, col]: p=(b2,kt,b0), col=(reim, j, ko);
        # b = g*32 + j*8 + 2*b2 + b0, k = kt*128 + ko
        d = br.results[c]["odump"].reshape(NGRP, 4, N2, 2, 2, 4, 128)
        arr = (
            d.transpose(4, 0, 5, 1, 3, 2, 6).reshape(2, BPC, NFFT).astype(np.float32)
        )
        out_re[bsl, :] = arr[0]
        out_im[bsl, :] = arr[1]
    return (out_re, out_im), br


def kernel(signal_re, signal_im):
    return run(signal_re, signal_im)[0]
